# revision 1
# baseline (speedup 1.0000x reference)
# Cross-attention kernel for Trainium2 (Bass/Tile), 8-core data-parallel.
#
# Reference computation (per batch element, B=8 -> one batch element per core):
#   q = x1 @ Wq.T + bq ; k = x2 @ Wk.T + bk ; v = x3 @ Wv.T + bv
#   out = softmax(q @ k.T) @ v          (no 1/sqrt(d) scale)
#
# Precision strategy (validated numerically against the fp32 reference,
# absmax rel err ~4e-3):
#   - q,k projections and q@k.T run as 3-pass bf16 hi/lo split matmuls
#     (hi = bf16(x), lo = bf16(x - hi); x@y ~= xh@yh + xh@yl + xl@yh),
#     accumulated in fp32 PSUM. Effective precision ~fp32 for the scores,
#     which matters because the unscaled scores have std ~46 and the softmax
#     is extremely sharp.
#   - v projection and attn@v run in plain bf16 (error contribution ~2e-3).
#   - softmax itself is fp32 (row max subtraction on-chip, exp on ScalarE,
#     normalization deferred to the output).
#
# Layout strategy per core (S=2048, C=1024, P=128):
#   - qT, kT computed directly transposed ([d, s], d on partitions) so the
#     score matmul contracts over d. All transposes (W, x, p) run on the DMA
#     xbar (2-byte transpose mode, SP HWDGE queue) -- they never touch the PE
#     or vector engines.
#   - v computed in natural [s, c] layout (stationary operand for attn@v).
#   - kT(hi/lo) and v stay resident in SBUF; qT(hi/lo) spills to a DRAM
#     scratch and streams back per 128-row query tile (ACT HWDGE queue, so
#     plain copies and xbar transposes live on different queues).
#   - p = exp(s - rowmax) transposed per sq-tile in one xbar DMA; the row sum
#     rides along via the activation accumulator, output normalized at the end.

from contextlib import ExitStack

import numpy as np

import concourse.bass as bass
import concourse.mybir as mybir
import concourse.tile as tile
from concourse import bacc
from concourse.bass_utils import run_bass_kernel_spmd

F32 = mybir.dt.float32
BF16 = mybir.dt.bfloat16
ADD = mybir.AluOpType.add
SUB = mybir.AluOpType.subtract
AX = mybir.AxisListType.X
EXP = mybir.ActivationFunctionType.Exp

B, S, C = 8, 2048, 1024
P = 128
NT_S = S // P  # 16 s-tiles
NT_C = C // P  # 8 c/d-tiles
CH = 512  # free-dim chunk (one fp32 PSUM bank)
NCH_S = S // CH  # 4
NCH_C = C // CH  # 2


def _emit(tc):
    nc = tc.nc

    x1 = nc.dram_tensor("x1", [S, C], F32, kind="ExternalInput").ap()
    x2 = nc.dram_tensor("x2", [S, C], F32, kind="ExternalInput").ap()
    x3 = nc.dram_tensor("x3", [S, C], F32, kind="ExternalInput").ap()
    Wq = nc.dram_tensor("Wq", [C, C], F32, kind="ExternalInput").ap()
    Wk = nc.dram_tensor("Wk", [C, C], F32, kind="ExternalInput").ap()
    Wv = nc.dram_tensor("Wv", [C, C], F32, kind="ExternalInput").ap()
    bq = nc.dram_tensor("bq", [C], F32, kind="ExternalInput").ap()
    bk = nc.dram_tensor("bk", [C], F32, kind="ExternalInput").ap()
    bv = nc.dram_tensor("bv", [C], F32, kind="ExternalInput").ap()
    out = nc.dram_tensor("out", [S, C], F32, kind="ExternalOutput").ap()

    es = ExitStack()
    with es:
        const = es.enter_context(tc.tile_pool(name="const", bufs=1))
        dram = es.enter_context(tc.tile_pool(name="dram", bufs=1, space="DRAM"))

        # biases: bq/bk as per-d-tile columns [128, 8]; bv broadcast [128, C]
        bq_sb = const.tile([P, NT_C], F32, tag="bq")
        nc.scalar.dma_start(out=bq_sb, in_=bq.rearrange("(t p) -> p t", p=P))
        bk_sb = const.tile([P, NT_C], F32, tag="bk")
        nc.scalar.dma_start(out=bk_sb, in_=bk.rearrange("(t p) -> p t", p=P))
        bv_sb = const.tile([P, C], F32, tag="bv")
        bv_bcast = bass.AP(tensor=bv.tensor, offset=bv.offset, ap=[[0, P], [1, C]])
        nc.scalar.dma_start(out=bv_sb, in_=bv_bcast)

        # DRAM scratch for spilled qT (hi/lo)
        qTh_d = dram.tile([NT_C, P, S], BF16, tag="qThd", name="qThd")
        qTl_d = dram.tile([NT_C, P, S], BF16, tag="qTld", name="qTld")

        def prep_w(W, wpool, split):
            """Load W [C,C] (rows d, cols c); produce W^T as one 3D tile
            [128c, NT_C(ct), C(d)] bf16 hi (and lo) via xbar transposes."""
            WhT = wpool.tile([P, NT_C, C], BF16, tag="WhT", name="WhT")
            WlT = wpool.tile([P, NT_C, C], BF16, tag="WlT", name="WlT") if split else None
            with tc.tile_pool(name="wstage", bufs=2) as ws:
                for dt in range(NT_C):
                    wnat = ws.tile([P, C], F32, tag="wnat", name="wnat")
                    weng = nc.gpsimd if dt % 2 == 0 else nc.scalar
                    weng.dma_start(out=wnat, in_=W[dt * P : (dt + 1) * P, :])
                    wh = ws.tile([P, C], BF16, tag="wh", name="wh")
                    nc.vector.tensor_copy(out=wh, in_=wnat)
                    nc.sync.dma_start(
                        out=WhT[:, :, dt * P : (dt + 1) * P], in_=wh, transpose=True
                    )
                    if split:
                        wl = ws.tile([P, C], BF16, tag="wl", name="wl")
                        nc.vector.tensor_tensor(out=wl, in0=wnat, in1=wh, op=SUB)
                        nc.sync.dma_start(
                            out=WlT[:, :, dt * P : (dt + 1) * P], in_=wl, transpose=True
                        )
            return WhT, WlT

        def prep_xT_chunk(x, s0, split, xs_pool, xt_pool):
            """Load x[s0:s0+CH, :] one s-tile at a time, split hi/lo, and xbar-
            transpose into [128c, NT_C(ct), CH(s)] bf16 tiles (hi, lo)."""
            nj = CH // P  # 4 s-tiles per chunk
            xhT = xt_pool.tile([P, NT_C, CH], BF16, tag="xhT", name="xhT")
            xlT = (
                xt_pool.tile([P, NT_C, CH], BF16, tag="xlT", name="xlT")
                if split
                else None
            )
            for j in range(nj):
                r0 = s0 + j * P
                xs = xs_pool.tile([P, C], F32, tag="xload", name="xload")
                xeng = nc.gpsimd if j % 2 == 0 else nc.scalar
                xeng.dma_start(out=xs, in_=x[r0 : r0 + P, :])
                xh = xs_pool.tile([P, C], BF16, tag="xh", name="xh")
                nc.vector.tensor_copy(out=xh, in_=xs)
                nc.sync.dma_start(
                    out=xhT[:, :, j * P : (j + 1) * P], in_=xh, transpose=True
                )
                if split:
                    xl = xs_pool.tile([P, C], BF16, tag="xl", name="xl")
                    nc.vector.tensor_tensor(out=xl, in0=xs, in1=xh, op=SUB)
                    nc.sync.dma_start(
                        out=xlT[:, :, j * P : (j + 1) * P], in_=xl, transpose=True
                    )
            return xhT, xlT

        def split_proj_mms(ps, xhT, xlT, WhT, WlT, dt):
            """Emit the 24 matmuls of a 3-pass split projection into psum ps."""
            n_mm = NT_C * 3
            i = 0
            for ct in range(NT_C):
                lw_h = WhT[:, ct, dt * P : (dt + 1) * P]
                nc.tensor.matmul(
                    ps, lw_h, xhT[:, ct, :], start=(i == 0), stop=(i == n_mm - 1)
                )
                i += 1
                nc.tensor.matmul(
                    ps, lw_h, xlT[:, ct, :], start=False, stop=(i == n_mm - 1)
                )
                i += 1
                lw_l = WlT[:, ct, dt * P : (dt + 1) * P]
                nc.tensor.matmul(
                    ps, lw_l, xhT[:, ct, :], start=False, stop=(i == n_mm - 1)
                )
                i += 1

        # ---------------- Phase Q: project qT (hi/lo) -> DRAM scratch --------
        with tc.tile_pool(name="wq", bufs=1) as wq_pool:
            WqhT, WqlT = prep_w(Wq, wq_pool, split=True)
            with (
                tc.tile_pool(name="qxs", bufs=2) as qxs,
                tc.tile_pool(name="qxt", bufs=2) as qxt,
                tc.tile_pool(name="qmmps", bufs=2, space="PSUM") as qmmps,
                tc.tile_pool(name="qst", bufs=3) as qst,
            ):
                nxt = prep_xT_chunk(x1, 0, True, qxs, qxt)
                for ich in range(NCH_S):
                    s0 = ich * CH
                    xhT, xlT = nxt
                    if ich + 1 < NCH_S:
                        nxt = prep_xT_chunk(x1, (ich + 1) * CH, True, qxs, qxt)
                    for dt in range(NT_C):
                        ps = qmmps.tile([P, CH], F32, tag="projps", name="projps")
                        split_proj_mms(ps, xhT, xlT, WqhT, WqlT, dt)
                        t = qst.tile([P, CH], F32, tag="projt", name="projt")
                        nc.vector.tensor_scalar_add(
                            out=t, in0=ps, scalar1=bq_sb[:, dt : dt + 1]
                        )
                        h = qst.tile([P, CH], BF16, tag="projh", name="projh")
                        nc.scalar.copy(out=h, in_=t)
                        l = qst.tile([P, CH], BF16, tag="projl", name="projl")
                        nc.vector.tensor_tensor(out=l, in0=t, in1=h, op=SUB)
                        nc.scalar.dma_start(out=qTh_d[dt, :, s0 : s0 + CH], in_=h)
                        nc.scalar.dma_start(out=qTl_d[dt, :, s0 : s0 + CH], in_=l)

        # ---------------- Phase K: project kT (hi/lo) -> resident SBUF -------
        res_k = es.enter_context(tc.tile_pool(name="resk", bufs=1))
        kTh = [
            res_k.tile([P, S], BF16, tag=f"kTh{i}", name=f"kTh{i}")
            for i in range(NT_C)
        ]
        kTl = [
            res_k.tile([P, S], BF16, tag=f"kTl{i}", name=f"kTl{i}")
            for i in range(NT_C)
        ]
        with tc.tile_pool(name="wk", bufs=1) as wk_pool:
            WkhT, WklT = prep_w(Wk, wk_pool, split=True)
            with (
                tc.tile_pool(name="kxs", bufs=2) as kxs,
                tc.tile_pool(name="kxt", bufs=2) as kxt,
                tc.tile_pool(name="kmmps", bufs=2, space="PSUM") as kmmps,
                tc.tile_pool(name="kst", bufs=3) as kst,
            ):
                nxt = prep_xT_chunk(x2, 0, True, kxs, kxt)
                for ich in range(NCH_S):
                    s0 = ich * CH
                    xhT, xlT = nxt
                    if ich + 1 < NCH_S:
                        nxt = prep_xT_chunk(x2, (ich + 1) * CH, True, kxs, kxt)
                    for dt in range(NT_C):
                        ps = kmmps.tile([P, CH], F32, tag="projps", name="kprojps")
                        split_proj_mms(ps, xhT, xlT, WkhT, WklT, dt)
                        t = kst.tile([P, CH], F32, tag="projt", name="kprojt")
                        nc.vector.tensor_scalar_add(
                            out=t, in0=ps, scalar1=bk_sb[:, dt : dt + 1]
                        )
                        h_sl = kTh[dt][:, s0 : s0 + CH]
                        nc.scalar.copy(out=h_sl, in_=t)
                        nc.vector.tensor_tensor(
                            out=kTl[dt][:, s0 : s0 + CH], in0=t, in1=h_sl, op=SUB
                        )

        # ---------------- Phase V: project v (natural [s, c]) -> resident ----
        res_v = es.enter_context(tc.tile_pool(name="resv", bufs=1))
        v_res = [
            res_v.tile([P, C], BF16, tag=f"v{i}", name=f"v{i}") for i in range(NT_S)
        ]
        with tc.tile_pool(name="wv", bufs=1) as wv_pool:
            WvhT, _ = prep_w(Wv, wv_pool, split=False)
            with (
                tc.tile_pool(name="vxs", bufs=2) as vxs,
                tc.tile_pool(name="vxt", bufs=2) as vxt,
                tc.tile_pool(name="vmmps", bufs=2, space="PSUM") as vmmps,
            ):
                nxt3 = prep_xT_chunk(x3, 0, False, vxs, vxt)
                for ich in range(NCH_S):
                    s0 = ich * CH
                    x3hT, _ = nxt3
                    if ich + 1 < NCH_S:
                        nxt3 = prep_xT_chunk(x3, (ich + 1) * CH, False, vxs, vxt)
                    for j in range(CH // P):  # s-tile within chunk
                        st = ich * (CH // P) + j
                        for cch in range(NCH_C):
                            ps = vmmps.tile([P, CH], F32, tag="vps", name="vps")
                            for ct in range(NT_C):
                                nc.tensor.matmul(
                                    ps,
                                    x3hT[:, ct, j * P : (j + 1) * P],
                                    WvhT[:, ct, cch * CH : (cch + 1) * CH],
                                    start=(ct == 0),
                                    stop=(ct == NT_C - 1),
                                )
                            nc.vector.tensor_tensor(
                                out=v_res[st][:, cch * CH : (cch + 1) * CH],
                                in0=ps,
                                in1=bv_sb[:, cch * CH : (cch + 1) * CH],
                                op=ADD,
                            )

        # ---------------- Attention ------------------------------------------
        with (
            tc.tile_pool(name="qstream", bufs=2) as qstream,
            tc.tile_pool(name="spsum", bufs=6, space="PSUM") as spsum,
            tc.tile_pool(name="opsum", bufs=2, space="PSUM") as opsum,
            tc.tile_pool(name="attn", bufs=2) as attn,
            tc.tile_pool(name="stats", bufs=4) as stats,
        ):
            for sq in range(NT_S):
                qh_t = qstream.tile([P, NT_C, P], BF16, tag="qh", name="qh")
                nc.scalar.dma_start(
                    out=qh_t,
                    in_=qTh_d[:, :, sq * P : (sq + 1) * P].rearrange("t p s -> p t s"),
                )
                ql_t = qstream.tile([P, NT_C, P], BF16, tag="ql", name="ql")
                nc.scalar.dma_start(
                    out=ql_t,
                    in_=qTl_d[:, :, sq * P : (sq + 1) * P].rearrange("t p s -> p t s"),
                )

                # scores: s[sq-tile, :] accumulated over d in 4 chunk banks
                ps_s = [
                    spsum.tile([P, CH], F32, tag="s", name=f"s{c}")
                    for c in range(NCH_S)
                ]
                cnt = [0] * NCH_S
                n_per = NT_C * 3
                for dt in range(NT_C):
                    qh_sl = qh_t[:, dt, :]
                    ql_sl = ql_t[:, dt, :]
                    for c in range(NCH_S):
                        nc.tensor.matmul(
                            ps_s[c],
                            qh_sl,
                            kTh[dt][:, c * CH : (c + 1) * CH],
                            start=(cnt[c] == 0),
                            stop=(cnt[c] == n_per - 1),
                        )
                        cnt[c] += 1
                    for c in range(NCH_S):
                        nc.tensor.matmul(
                            ps_s[c],
                            qh_sl,
                            kTl[dt][:, c * CH : (c + 1) * CH],
                            start=False,
                            stop=(cnt[c] == n_per - 1),
                        )
                        cnt[c] += 1
                    for c in range(NCH_S):
                        nc.tensor.matmul(
                            ps_s[c],
                            ql_sl,
                            kTh[dt][:, c * CH : (c + 1) * CH],
                            start=False,
                            stop=(cnt[c] == n_per - 1),
                        )
                        cnt[c] += 1

                # softmax (fp32, row-wise over the free dim)
                mx = stats.tile([P, NCH_S], F32, tag="mx", name="mx")
                for c in range(NCH_S):
                    nc.vector.reduce_max(out=mx[:, c : c + 1], in_=ps_s[c], axis=AX)
                negmax = stats.tile([P, 1], F32, tag="negmax", name="negmax")
                nc.vector.reduce_max(out=negmax, in_=mx, axis=AX, negate=True)

                p_sb = attn.tile([P, S], BF16, tag="p", name="p")
                sums = stats.tile([P, NCH_S], F32, tag="sums", name="sums")
                for c in range(NCH_S):
                    nc.scalar.activation(
                        out=p_sb[:, c * CH : (c + 1) * CH],
                        in_=ps_s[c],
                        func=EXP,
                        bias=negmax,
                        scale=1.0,
                        accum_out=sums[:, c : c + 1],
                    )
                rs = stats.tile([P, 1], F32, tag="rs", name="rs")
                nc.vector.reduce_sum(out=rs, in_=sums, axis=AX)
                rinv = stats.tile([P, 1], F32, tag="rinv", name="rinv")
                nc.vector.reciprocal(out=rinv, in_=rs)

                # transpose p for attn @ v: one xbar DMA per sq-tile
                pT = attn.tile([P, NT_S, P], BF16, tag="pT", name="pT")
                nc.sync.dma_start(out=pT, in_=p_sb, transpose=True)

                # attn @ v, accumulate over sk tiles; normalize; store
                ps_o = [
                    opsum.tile([P, CH], F32, tag="o", name=f"o{c}")
                    for c in range(NCH_C)
                ]
                for skt in range(NT_S):
                    for cch in range(NCH_C):
                        nc.tensor.matmul(
                            ps_o[cch],
                            pT[:, skt, :],
                            v_res[skt][:, cch * CH : (cch + 1) * CH],
                            start=(skt == 0),
                            stop=(skt == NT_S - 1),
                        )
                o_sb = attn.tile([P, C], F32, tag="osb", name="osb")
                for cch in range(NCH_C):
                    nc.vector.tensor_scalar_mul(
                        out=o_sb[:, cch * CH : (cch + 1) * CH],
                        in0=ps_o[cch],
                        scalar1=rinv,
                    )
                nc.scalar.dma_start(out=out[sq * P : (sq + 1) * P, :], in_=o_sb)


_BUILT = {}


def _build():
    if "nc" not in _BUILT:
        nc = bacc.Bacc(
            "TRN2",
            target_bir_lowering=False,
            debug=False,
            num_devices=B,
        )
        with tile.TileContext(nc) as tc:
            _emit(tc)
        nc.compile()
        _BUILT["nc"] = nc
    return _BUILT["nc"]


def kernel_with_results(trace=False, **inputs):
    nc = _build()
    in_maps = []
    for i in range(B):
        in_maps.append(
            {
                "x1": np.ascontiguousarray(inputs["x1"][i], dtype=np.float32),
                "x2": np.ascontiguousarray(inputs["x2"][i], dtype=np.float32),
                "x3": np.ascontiguousarray(inputs["x3"][i], dtype=np.float32),
                "Wq": np.ascontiguousarray(inputs["Wq"], dtype=np.float32),
                "Wk": np.ascontiguousarray(inputs["Wk"], dtype=np.float32),
                "Wv": np.ascontiguousarray(inputs["Wv"], dtype=np.float32),
                "bq": np.ascontiguousarray(inputs["bq"], dtype=np.float32),
                "bk": np.ascontiguousarray(inputs["bk"], dtype=np.float32),
                "bv": np.ascontiguousarray(inputs["bv"], dtype=np.float32),
            }
        )
    res = run_bass_kernel_spmd(nc, in_maps, core_ids=list(range(B)), trace=trace)
    outs = np.stack([r["out"] for r in res.results], axis=0).astype(np.float32)
    return outs, res


def kernel(**inputs):
    outs, _ = kernel_with_results(trace=False, **inputs)
    return outs



# revision 22
# speedup vs baseline: 1.8747x; 1.8747x over previous
# Cross-attention kernel for Trainium2 (Bass/Tile), 8-core data-parallel.
#
# Reference (per batch element, B=8 -> one batch element per core):
#   q = x1 @ Wq.T + bq ; k = x2 @ Wk.T + bk
#   out = softmax(q @ k.T) @ (x3 @ Wv.T + bv)     (no 1/sqrt(d) scale)
#
# Algorithmic restructure: softmax(q @ k.T) == softmax(t @ x2.T) where
#   t = x1 @ M + bq @ Wk,  M = Wq.T @ Wk
# because q @ k.T = t @ x2.T + (q . bk) 1^T and the (q . bk) term is
# constant along each softmax row. k is never computed; x2^T itself is the
# "key" matrix.
#
# Precision: the score path runs entirely in float32r (fp32 rounded to 11
# explicit mantissa bits by the PE) which the PE processes at bf16 speed
# (1 cycle/row for moving dim >= 256) -- no hi/lo split matmuls needed.
# Validated end-to-end in numpy emulation: absmax rel err ~8.7e-3 (tol 2e-2).
# v projection and attn @ v run in plain bf16 as before (~2e-3 contribution).
#
# Layout per core (S=2048, C=1024, P=128):
#   - M[c,c'] = sum_d Wq[d,c] Wk[d,c']: both weights consumed in NATURAL
#     layout (d on partitions) -- no weight transposes at all.
#   - x1^T, x2^T (c on partitions) built via bf16 hi/lo xbar transposes
#     reassembled to f32r with one DVE/Pool add (hi+lo).
#   - tT[c',s] = sum_c M[c,c'] x1T[c,s] (+ b_t[c'] bias, fused into the
#     Act-engine PSUM evacuation) -> spilled to DRAM, streamed back per
#     128-row query tile during attention.
#   - x2T stays resident in SBUF (8MB f32r); v resident bf16 (4MB).
#   - softmax: fp32 row max on DVE, exp on Act (bias=-max, accum row sums),
#     p stored bf16, transposed via xbar, normalization after attn @ v.

from contextlib import ExitStack

import numpy as np

import concourse.bass as bass
import concourse.mybir as mybir
import concourse.tile as tile
from concourse import bacc
from concourse.bass_utils import run_bass_kernel_spmd

F32 = mybir.dt.float32
F32R = mybir.dt.float32r
BF16 = mybir.dt.bfloat16
ADD = mybir.AluOpType.add
SUB = mybir.AluOpType.subtract
AX = mybir.AxisListType.X
EXP = mybir.ActivationFunctionType.Exp
IDENT = mybir.ActivationFunctionType.Identity

B, S, C = 8, 2048, 1024
P = 128
NT_S = S // P  # 16 s-tiles
NT_C = C // P  # 8 c/d-tiles
CH = 512  # free-dim chunk (one fp32 PSUM bank)
NCH_S = S // CH  # 4
NCH_C = C // CH  # 2


def _emit(tc):
    nc = tc.nc

    x1 = nc.dram_tensor("x1", [S, C], F32, kind="ExternalInput").ap()
    x2 = nc.dram_tensor("x2", [S, C], F32, kind="ExternalInput").ap()
    x3 = nc.dram_tensor("x3", [S, C], F32, kind="ExternalInput").ap()
    Wq = nc.dram_tensor("Wq", [C, C], F32, kind="ExternalInput").ap()
    Wk = nc.dram_tensor("Wk", [C, C], F32, kind="ExternalInput").ap()
    Wv = nc.dram_tensor("Wv", [C, C], F32, kind="ExternalInput").ap()
    bq = nc.dram_tensor("bq", [C], F32, kind="ExternalInput").ap()
    bv = nc.dram_tensor("bv", [C], F32, kind="ExternalInput").ap()
    out = nc.dram_tensor("out", [S, C], F32, kind="ExternalOutput").ap()

    es = ExitStack()
    with es:
        const = es.enter_context(tc.tile_pool(name="const", bufs=1))
        dram = es.enter_context(tc.tile_pool(name="dram", bufs=1, space="DRAM"))
        res_x2 = es.enter_context(tc.tile_pool(name="resx2", bufs=1))

        # bq as per-d-tile columns [128, 8] -> f32r, duplicated to [128,8,2]
        bq_sb = const.tile([P, NT_C], F32, tag="bq")
        nc.scalar.dma_start(out=bq_sb, in_=bq.rearrange("(t p) -> p t", p=P))
        bq2 = const.tile([P, NT_C, 2], F32R, tag="bq2")
        nc.vector.tensor_copy(out=bq2[:, :, 0:1], in_=bq_sb.unsqueeze(-1))
        nc.vector.tensor_copy(out=bq2[:, :, 1:2], in_=bq_sb.unsqueeze(-1))
        # bv broadcast [128, C]
        bv_sb = const.tile([P, C], F32, tag="bv")
        bv_bcast = bass.AP(tensor=bv.tensor, offset=bv.offset, ap=[[0, P], [1, C]])
        nc.scalar.dma_start(out=bv_sb, in_=bv_bcast)
        # b_t = bq @ Wk, per c'-tile column [128, 8] (filled in phase A)
        b_t_sb = const.tile([P, NT_C], F32, tag="bt")

        # resident keys: x2T f32r [c-tile][128, S]
        x2T = [
            res_x2.tile([P, S], F32R, tag=f"x2T{i}", name=f"x2T{i}")
            for i in range(NT_C)
        ]
        # DRAM scratch for spilled tT (f32r)
        tT_d = dram.tile([NT_C, P, S], F32R, tag="tTd", name="tTd")

        def stage_xT_f32r(x, s0, ld, hl, tp, dst_tiles, dst_off, ld_eng=None):
            """Stage x[s0:s0+CH, :] -> f32r transposed chunks.

            dst_tiles: list of 8 [P, S]-f32r tiles (write at column dst_off)
            or a single [P, NT_C, CH] tile when dst_off is None.
            """
            xhT = tp.tile([P, NT_C, CH], BF16, tag="xhT", name="xhT")
            xlT = tp.tile([P, NT_C, CH], BF16, tag="xlT", name="xlT")
            for j in range(CH // P):
                r0 = s0 + j * P
                xs = ld.tile([P, C], F32, tag="xload", name="xload")
                if ld_eng is None:
                    xeng = nc.gpsimd if j % 2 == 0 else nc.scalar
                else:
                    xeng = ld_eng
                xeng.dma_start(out=xs, in_=x[r0 : r0 + P, :])
                xh = hl.tile([P, C], BF16, tag="xh", name="xh")
                nc.scalar.copy(out=xh, in_=xs)
                nc.sync.dma_start(
                    out=xhT[:, :, j * P : (j + 1) * P], in_=xh, transpose=True
                )
                xl = hl.tile([P, C], BF16, tag="xl", name="xl")
                nc.vector.tensor_tensor(out=xl, in0=xs, in1=xh, op=SUB)
                nc.sync.dma_start(
                    out=xlT[:, :, j * P : (j + 1) * P], in_=xl, transpose=True
                )
            for ct in range(NT_C):
                eng = nc.gpsimd if ct % 2 == 0 else nc.vector
                if dst_off is None:
                    o = dst_tiles[:, ct, :]
                else:
                    o = dst_tiles[ct][:, dst_off : dst_off + CH]
                eng.tensor_tensor(
                    out=o, in0=xhT[:, ct, :], in1=xlT[:, ct, :], op=ADD
                )

        # ---------------- Phase A: M = Wq^T @ Wk, b_t = bq @ Wk -------------
        wv_pool = es.enter_context(tc.tile_pool(name="wv", bufs=1))
        WvhT = wv_pool.tile([P, NT_C, C], BF16, tag="WvhT", name="WvhT")
        res_M_cm = tc.tile_pool(name="resM", bufs=1)
        res_M = res_M_cm.__enter__()
        M_r = [
            res_M.tile([P, C], F32R, tag=f"M{i}", name=f"M{i}") for i in range(NT_C)
        ]
        with tc.tile_pool(name="wpool", bufs=1) as wpool:
            wq_r = [
                wpool.tile([P, C], F32R, tag=f"wq{i}", name=f"wq{i}")
                for i in range(NT_C)
            ]
            wk_r = [
                wpool.tile([P, C], F32R, tag=f"wk{i}", name=f"wk{i}")
                for i in range(NT_C)
            ]
            with tc.tile_pool(name="wstage", bufs=4) as ws:
                for dt in range(NT_C):
                    for i, (W, dst) in enumerate(
                        [(Wq, wq_r), (Wk, wk_r)]
                    ):
                        wnat = ws.tile([P, C], F32, tag="wnat", name="wnat")
                        nc.gpsimd.dma_start(out=wnat, in_=W[dt * P : (dt + 1) * P, :])
                        nc.vector.tensor_copy(out=dst[dt], in_=wnat)

            # dt-outer accumulation over 8 PSUM banks (two ct-half passes):
            # the first matmuls only need wq[0]/wk[0], so the PE starts as
            # soon as the first weight tiles land instead of after all 16.
            with tc.tile_pool(name="mps", bufs=8, space="PSUM") as mps:
                for half in range(2):
                    cts = range(half * 4, half * 4 + 4)
                    ps_h = {
                        (ct, cch): mps.tile([P, CH], F32, tag="mps", name="mps")
                        for ct in cts
                        for cch in range(NCH_C)
                    }
                    for dt in range(NT_C):
                        for ct in cts:
                            for cch in range(NCH_C):
                                nc.tensor.matmul(
                                    ps_h[(ct, cch)],
                                    wq_r[dt][:, ct * P : (ct + 1) * P],
                                    wk_r[dt][:, cch * CH : (cch + 1) * CH],
                                    start=(dt == 0),
                                    stop=(dt == NT_C - 1),
                                )
                    for ct in cts:
                        for cch in range(NCH_C):
                            nc.vector.tensor_copy(
                                out=M_r[ct][:, cch * CH : (cch + 1) * CH],
                                in_=ps_h[(ct, cch)],
                            )
            with tc.tile_pool(name="btps", bufs=2, space="PSUM") as btps:
                for ct in range(NT_C):
                    psb = btps.tile([P, 2], F32, tag="btps", name="btps")
                    for dt in range(NT_C):
                        nc.tensor.matmul(
                            psb,
                            wk_r[dt][:, ct * P : (ct + 1) * P],
                            bq2[:, dt, :],
                            start=(dt == 0),
                            stop=(dt == NT_C - 1),
                        )
                    nc.vector.tensor_copy(
                        out=b_t_sb[:, ct : ct + 1], in_=psb[:, 0:1]
                    )

        # ---------------- Phase B: x1T staging + tT -> DRAM ------------------
        with (
            tc.tile_pool(name="ld", bufs=3) as ld,
            tc.tile_pool(name="hl", bufs=3) as hl,
            tc.tile_pool(name="tp", bufs=1) as tp,
            tc.tile_pool(name="x1r", bufs=2) as x1rp,
            tc.tile_pool(name="tps", bufs=2, space="PSUM") as tps,
            tc.tile_pool(name="tst", bufs=2) as tst,
        ):

            def stage_x1(ich):
                x1r = x1rp.tile([P, NT_C, CH], F32R, tag="x1r", name="x1r")
                # x1 loads go on the (otherwise idle) DVE HWDGE queue so they
                # are not stuck behind the Wq/Wk/Wv loads.
                stage_xT_f32r(x1, ich * CH, ld, hl, tp, x1r, None, ld_eng=nc.scalar)
                return x1r

            def prep_wv():
                # Emitted after the first x1 chunk is staged so Wv's loads
                # and rounds don't head-of-line-block x1 on the Act queue.
                with tc.tile_pool(name="wvs", bufs=2) as wvs:
                    for dt in range(NT_C):
                        wnat = wvs.tile([P, C], F32, tag="wnat", name="wnat")
                        nc.gpsimd.dma_start(
                            out=wnat, in_=Wv[dt * P : (dt + 1) * P, :]
                        )
                        wh = wvs.tile([P, C], BF16, tag="wh", name="wh")
                        nc.scalar.copy(out=wh, in_=wnat)
                        nc.sync.dma_start(
                            out=WvhT[:, :, dt * P : (dt + 1) * P],
                            in_=wh,
                            transpose=True,
                        )

            nxt = stage_x1(0)
            for ich in range(NCH_S):
                s0 = ich * CH
                x1r = nxt
                if ich + 1 < NCH_S:
                    nxt = stage_x1(ich + 1)
                if ich == 1:
                    prep_wv()
                for cpt in range(NT_C):
                    ps = tps.tile([P, CH], F32, tag="tps", name="tps")
                    for ct in range(NT_C):
                        nc.tensor.matmul(
                            ps,
                            M_r[ct][:, cpt * P : (cpt + 1) * P],
                            x1r[:, ct, :],
                            start=(ct == 0),
                            stop=(ct == NT_C - 1),
                        )
                    t = tst.tile([P, CH], F32R, tag="tst", name="tst")
                    nc.scalar.activation(
                        out=t,
                        in_=ps,
                        func=IDENT,
                        bias=b_t_sb[:, cpt : cpt + 1],
                        scale=1.0,
                    )
                    nc.scalar.dma_start(out=tT_d[cpt, :, s0 : s0 + CH], in_=t)

        res_M_cm.__exit__(None, None, None)  # free M before phase C pools open

        # -------- Phase C: v (bf16) with x2 -> x2T staging overlapped --------
        res_v = es.enter_context(tc.tile_pool(name="resv", bufs=1))
        v_res = [
            res_v.tile([P, C], BF16, tag=f"v{i}", name=f"v{i}") for i in range(NT_S)
        ]
        with (
            tc.tile_pool(name="vld", bufs=3) as vld,
            tc.tile_pool(name="vtp", bufs=2) as vtp,
            tc.tile_pool(name="vps", bufs=2, space="PSUM") as vps,
            tc.tile_pool(name="ld2", bufs=3) as ld2,
            tc.tile_pool(name="hl2", bufs=3) as hl2,
            tc.tile_pool(name="tp2", bufs=1) as tp2,
        ):

            def stage_x3(ich):
                x3hT = vtp.tile([P, NT_C, CH], BF16, tag="x3hT", name="x3hT")
                for j in range(CH // P):
                    r0 = ich * CH + j * P
                    xs = vld.tile([P, C], F32, tag="x3load", name="x3load")
                    xeng = nc.gpsimd if j % 2 == 0 else nc.scalar
                    xeng.dma_start(out=xs, in_=x3[r0 : r0 + P, :])
                    xh = vld.tile([P, C], BF16, tag="x3h", name="x3h")
                    nc.scalar.copy(out=xh, in_=xs)
                    nc.sync.dma_start(
                        out=x3hT[:, :, j * P : (j + 1) * P],
                        in_=xh,
                        transpose=True,
                    )
                return x3hT

            nxt3 = stage_x3(0)
            for ich in range(NCH_S):
                x3hT = nxt3
                if ich + 1 < NCH_S:
                    nxt3 = stage_x3(ich + 1)
                # keys: x2 -> x2T (f32r, resident); consumed by attention only
                stage_xT_f32r(x2, ich * CH, ld2, hl2, tp2, x2T, ich * CH)
                for j in range(CH // P):
                    st = ich * (CH // P) + j
                    for cch in range(NCH_C):
                        ps = vps.tile([P, CH], F32, tag="vps", name="vps")
                        for ct in range(NT_C):
                            nc.tensor.matmul(
                                ps,
                                x3hT[:, ct, j * P : (j + 1) * P],
                                WvhT[:, ct, cch * CH : (cch + 1) * CH],
                                start=(ct == 0),
                                stop=(ct == NT_C - 1),
                            )
                        nc.vector.tensor_tensor(
                            out=v_res[st][:, cch * CH : (cch + 1) * CH],
                            in0=ps,
                            in1=bv_sb[:, cch * CH : (cch + 1) * CH],
                            op=ADD,
                        )

        # ---------------- Phase D: attention ---------------------------------
        with (
            tc.tile_pool(name="qstream", bufs=2) as qstream,
            tc.tile_pool(name="spsum", bufs=6, space="PSUM") as spsum,
            tc.tile_pool(name="opsum", bufs=2, space="PSUM") as opsum,
            tc.tile_pool(name="attn", bufs=2) as attn,
            tc.tile_pool(name="stats", bufs=4) as stats,
        ):
            # Software-pipelined with lag-1: attn @ v for tile sq-1 is emitted
            # AFTER the score matmuls for tile sq, so the PE never waits on
            # the softmax chain (it runs on DVE/Act/SP under the next tile's
            # score matmuls).
            pend = None  # (pT, rinv, sq) awaiting attn @ v

            def emit_attn(pT, rinv, sq):
                ps_o = [
                    opsum.tile([P, CH], F32, tag="o", name=f"o{c}")
                    for c in range(NCH_C)
                ]
                for skt in range(NT_S):
                    for cch in range(NCH_C):
                        nc.tensor.matmul(
                            ps_o[cch],
                            pT[:, skt, :],
                            v_res[skt][:, cch * CH : (cch + 1) * CH],
                            start=(skt == 0),
                            stop=(skt == NT_S - 1),
                        )
                o_sb = attn.tile([P, C], F32, tag="osb", name="osb")
                for cch in range(NCH_C):
                    nc.vector.tensor_scalar_mul(
                        out=o_sb[:, cch * CH : (cch + 1) * CH],
                        in0=ps_o[cch],
                        scalar1=rinv,
                    )
                nc.scalar.dma_start(out=out[sq * P : (sq + 1) * P, :], in_=o_sb)

            tq_next = qstream.tile([P, NT_C, P], F32R, tag="tq", name="tq")
            nc.scalar.dma_start(
                out=tq_next, in_=tT_d[:, :, 0:P].rearrange("t p s -> p t s")
            )
            for sq in range(NT_S):
                tq = tq_next
                if sq + 1 < NT_S:
                    tq_next = qstream.tile([P, NT_C, P], F32R, tag="tq", name="tq")
                    nc.scalar.dma_start(
                        out=tq_next,
                        in_=tT_d[:, :, (sq + 1) * P : (sq + 2) * P].rearrange(
                            "t p s -> p t s"
                        ),
                    )

                # scores: 4 chunk banks, accumulate over c'-tiles
                ps_s = [
                    spsum.tile([P, CH], F32, tag="s", name=f"s{c}")
                    for c in range(NCH_S)
                ]
                mx = stats.tile([P, NCH_S], F32, tag="mx", name="mx")
                for c in range(NCH_S):
                    for cpt in range(NT_C):
                        nc.tensor.matmul(
                            ps_s[c],
                            tq[:, cpt, :],
                            x2T[cpt][:, c * CH : (c + 1) * CH],
                            start=(cpt == 0),
                            stop=(cpt == NT_C - 1),
                        )
                    nc.vector.reduce_max(out=mx[:, c : c + 1], in_=ps_s[c], axis=AX)

                if pend is not None:
                    emit_attn(*pend)

                negmax = stats.tile([P, 1], F32, tag="negmax", name="negmax")
                nc.vector.reduce_max(out=negmax, in_=mx, axis=AX, negate=True)

                p_sb = attn.tile([P, S], BF16, tag="p", name="p")
                sums = stats.tile([P, NCH_S], F32, tag="sums", name="sums")
                for c in range(NCH_S):
                    nc.scalar.activation(
                        out=p_sb[:, c * CH : (c + 1) * CH],
                        in_=ps_s[c],
                        func=EXP,
                        bias=negmax,
                        scale=1.0,
                        accum_out=sums[:, c : c + 1],
                    )
                rs = stats.tile([P, 1], F32, tag="rs", name="rs")
                nc.vector.reduce_sum(out=rs, in_=sums, axis=AX)
                rinv = stats.tile([P, 1], F32, tag="rinv", name="rinv")
                nc.vector.reciprocal(out=rinv, in_=rs)

                pT = attn.tile([P, NT_S, P], BF16, tag="pT", name="pT")
                nc.sync.dma_start(out=pT, in_=p_sb, transpose=True)
                pend = (pT, rinv, sq)

            emit_attn(*pend)


_BUILT = {}


def _build():
    if "nc" not in _BUILT:
        nc = bacc.Bacc(
            "TRN2",
            target_bir_lowering=False,
            debug=False,
            num_devices=B,
        )
        with tile.TileContext(nc) as tc:
            _emit(tc)
        nc.compile()
        _BUILT["nc"] = nc
    return _BUILT["nc"]


def kernel_with_results(trace=False, **inputs):
    nc = _build()
    in_maps = []
    for i in range(B):
        in_maps.append(
            {
                "x1": np.ascontiguousarray(inputs["x1"][i], dtype=np.float32),
                "x2": np.ascontiguousarray(inputs["x2"][i], dtype=np.float32),
                "x3": np.ascontiguousarray(inputs["x3"][i], dtype=np.float32),
                "Wq": np.ascontiguousarray(inputs["Wq"], dtype=np.float32),
                "Wk": np.ascontiguousarray(inputs["Wk"], dtype=np.float32),
                "Wv": np.ascontiguousarray(inputs["Wv"], dtype=np.float32),
                "bq": np.ascontiguousarray(inputs["bq"], dtype=np.float32),
                "bv": np.ascontiguousarray(inputs["bv"], dtype=np.float32),
            }
        )
    res = run_bass_kernel_spmd(nc, in_maps, core_ids=list(range(B)), trace=trace)
    outs = np.stack([r["out"] for r in res.results], axis=0).astype(np.float32)
    return outs, res


def kernel(**inputs):
    outs, _ = kernel_with_results(trace=False, **inputs)
    return outs
